# revision 24
# baseline (speedup 1.0000x reference)
"""Causal multi-head attention block (qkv proj + attention + out proj) on 8
Trainium2 NeuronCores.

Sharding: core c = 2*b + hg handles batch b (of 4) and head-group hg (8 of 16
heads).  Each core computes qkv for its heads, causal attention, and a partial
out-projection (its 512 rows of w_out); the host sums the two head-group
partials per batch.

Per-core layout (all matmuls fp32r):
  - x [T, DIM] is PE-transposed into xT [DIM, T] per t-quarter; Q^T/K^T come
    out of the projection as head-pair tiles [128 = 2 heads x 64, t]; V in
    natural [t, c] layout augmented with a ones column per head (V_aug), so
    P @ V_aug accumulates the numerator and the softmax denominator together
    (softmax runs without max-subtraction: scores ~ N(0,1), exp safe in fp32).
  - scores are computed transposed, S^T[k, q], two heads concurrently via PE
    row tiling (K=64 each) into one two-bank PSUM tile; exp (scale fused) is
    one ACT op per pair, narrowed on diagonal blocks; causal masking is a 0/1
    multiply on one 128-wide strip + a zero-fill left of it.
  - normalization: denominators DMA into an [8, 512] tile, one batched DVE
    reciprocal per q-block, DMA partition-broadcast, in-place DVE multiply.
  - emission interleaves qkv quarter q+1 and out_proj q-1 (dense PE work)
    into attention block q (ACT-bound) to keep the PE busy and HAM-warm.
"""

import sys

if "/opt/trn_rl_repo" not in sys.path:
    sys.path.insert(0, "/opt/trn_rl_repo")

import numpy as np

import concourse.bass as bass
import concourse.mybir as mybir
import concourse.tile as tile
from concourse import bacc
from concourse.masks import make_identity
from concourse.bass_utils import run_bass_kernel_spmd

DIM = 1024
N_HEAD = 16
HD = 64
B, T = 4, 2048
HG = 8          # heads per core
CQ = HG * HD    # 512 feature columns per group
NCORES = 8
NT = T // 128   # 16 t-subtiles
NQ = T // 512   # 4 quarters / q-blocks

f32 = mybir.dt.float32
f32r = mybir.dt.float32r
Exp = mybir.ActivationFunctionType.Exp


def build_nc():
    nc = bacc.Bacc(None, target_bir_lowering=False)
    x_d = nc.declare_dram_parameter("x", [T, DIM], f32, isOutput=False)
    wqk_d = nc.declare_dram_parameter("wqk", [DIM, 2 * CQ], f32, isOutput=False)
    wv_d = nc.declare_dram_parameter("wv", [DIM, CQ], f32, isOutput=False)
    wo_d = nc.declare_dram_parameter("wo", [CQ, DIM], f32, isOutput=False)
    mv_d = nc.declare_dram_parameter("maskv", [128, NT], f32, isOutput=False)
    out_d = nc.declare_dram_parameter("out", [T, DIM], f32, isOutput=True)

    with tile.TileContext(nc) as tc:
        with tc.tile_pool(name="pp", bufs=1) as pp, \
             tc.tile_pool(name="qtp", bufs=2) as qtp, \
             tc.tile_pool(name="xrow", bufs=1) as xrow_p, \
             tc.tile_pool(name="xT", bufs=1) as xT_p, \
             tc.tile_pool(name="p_p", bufs=3) as p_p, \
             tc.tile_pool(name="at_p", bufs=2) as at_p, \
             tc.tile_pool(name="den_p", bufs=1) as den_p, \
             tc.tile_pool(name="rec_p", bufs=1) as rec_p, \
             tc.tile_pool(name="bcs_p", bufs=1) as bcs_p, \
             tc.tile_pool(name="out_p", bufs=2) as out_p, \
             tc.tile_pool(name="dram_p", bufs=2, space="DRAM") as dram_p, \
             tc.tile_pool(name="ps_aux", bufs=2, space="PSUM") as ps_aux, \
             tc.tile_pool(name="ps_s", bufs=2, space="PSUM") as ps_s, \
             tc.tile_pool(name="ps_pv", bufs=1, space="PSUM") as ps_pv:

            # ---- constants ----
            ident32 = pp.tile([128, 128], f32, name="ident32", tag="ident32")
            make_identity(nc, ident32)
            ident = pp.tile([128, 128], f32r, name="ident", tag="ident")
            nc.vector.tensor_copy(ident, ident32)
            # one 128x128 causal strip: keep where q_local >= k_local
            dstrip = pp.tile([128, 128], f32, name="dstrip", tag="dstrip")
            nc.gpsimd.memset(dstrip, 1.0)
            nc.gpsimd.affine_select(
                out=dstrip, in_=dstrip, compare_op=mybir.AluOpType.is_ge,
                fill=0.0, base=0, pattern=[[1, 128]], channel_multiplier=-1)
            zerosr = pp.tile([128, 384], f32r, name="zerosr", tag="zerosr")
            nc.vector.memset(zerosr.bitcast(f32), 0.0)
            onescol = pp.tile([128, HG], f32, name="onescol", tag="onescol")
            nc.vector.memset(onescol, 1.0)
            mv_sb = pp.tile([128, NT], f32, name="maskv_sb", tag="maskv_sb")
            nc.sync.dma_start(out=mv_sb, in_=mv_d[:, :])

            # ---- persistent tensors ----
            kt = [pp.tile([128, T], f32r, name=f"kt{m}", tag=f"kt{m}") for m in range(4)]
            vaug = [pp.tile([128, HG * 65], f32r, name=f"vaug{t}", tag=f"vaug{t}")
                    for t in range(NT)]
            wo_sb = [pp.tile([128, DIM], f32r, name=f"wo{m}", tag=f"wo{m}")
                     for m in range(4)]
            wqk_sb = [pp.tile([128, 2 * CQ], f32r, name=f"wqk{k}", tag=f"wqk{k}")
                      for k in range(8)]
            wv_sb = [pp.tile([128, CQ], f32r, name=f"wv{k}", tag=f"wv{k}")
                     for k in range(8)]
            for m in range(4):
                nc.sync.dma_start(out=wo_sb[m],
                                  in_=wo_d[m * 128:(m + 1) * 128, :].bitcast(f32r))
            for k in range(8):
                nc.sync.dma_start(out=wqk_sb[k],
                                  in_=wqk_d[k * 128:(k + 1) * 128, :].bitcast(f32r))
                nc.sync.dma_start(out=wv_sb[k],
                                  in_=wv_d[k * 128:(k + 1) * 128, :].bitcast(f32r))

            qt_cur = {}    # quarter -> [4 pair tiles [128, 512]]
            ats_cur = {}   # qb -> [4 pair tiles [128, 512]]

            # ---------- qkv quarter units (each closure ~1-2 us of PE) ----------
            def qkv_units(q):
                units = []
                xts = [xT_p.tile([128, 512], f32r, name=f"xt{kb}", tag=f"xt{kb}")
                       for kb in range(8)]

                def xt_unit(ti):
                    # load 128 rows of x, PE-transpose into the 8 xT tiles
                    xr = xrow_p.tile([128, DIM], f32r, name="xr", tag="xr")
                    t0 = (q * 4 + ti) * 128
                    nc.sync.dma_start(out=xr, in_=x_d[t0:t0 + 128, :].bitcast(f32r))
                    for kb in range(8):
                        pst = ps_aux.tile([128, 128], f32r, name="pst", tag="aux")
                        nc.tensor.transpose(
                            pst, xr[:, kb * 128:(kb + 1) * 128], ident)
                        nc.vector.tensor_copy(
                            xts[kb][:, ti * 128:(ti + 1) * 128], pst)
                for ti in range(4):
                    units.append(lambda ti=ti: xt_unit(ti))

                qt_cur[q] = [None] * 4

                def qk_unit(m):
                    pq = ps_aux.tile([128, 512], f32, name="mm", tag="aux")
                    for kb in range(8):
                        nc.tensor.matmul(
                            pq, wqk_sb[kb][:, m * 128:(m + 1) * 128], xts[kb],
                            start=(kb == 0), stop=(kb == 7))
                    if m < 4:
                        qtile = qtp.tile([128, 512], f32r, name=f"qt{m}", tag=f"qt{m}")
                        nc.vector.tensor_copy(qtile, pq)
                        qt_cur[q][m] = qtile
                    else:
                        nc.vector.tensor_copy(
                            kt[m - 4][:, q * 512:(q + 1) * 512], pq)
                for m in range(8):
                    units.append(lambda m=m: qk_unit(m))

                def v_unit(ti):
                    pv = ps_aux.tile([128, 512], f32, name="mm", tag="aux")
                    for kb in range(8):
                        nc.tensor.matmul(
                            pv, xts[kb][:, ti * 128:(ti + 1) * 128], wv_sb[kb],
                            start=(kb == 0), stop=(kb == 7))
                    vt = vaug[q * 4 + ti]
                    vt3 = vt.rearrange("p (h w) -> p h w", w=65)
                    nc.vector.tensor_copy(
                        vt3[:, :, 0:64], pv.rearrange("p (h w) -> p h w", w=64))
                    nc.vector.tensor_copy(
                        vt3[:, :, 64:65], onescol.rearrange("p (h w) -> p h w", w=1))
                    nc.vector.tensor_scalar_mul(
                        vt, vt, mv_sb[:, (q * 4 + ti):(q * 4 + ti + 1)])
                for ti in range(4):
                    units.append(lambda ti=ti: v_unit(ti))
                return units

            # ---------- out_proj units for one q-block ----------
            def outproj_units(qb):
                units = []

                def op_unit(ti, nb):
                    ats = ats_cur[qb]
                    po = ps_aux.tile([128, 512], f32, name="mm", tag="aux")
                    for m in range(4):
                        nc.tensor.matmul(
                            po, ats[m][:, ti * 128:(ti + 1) * 128],
                            wo_sb[m][:, nb * 512:(nb + 1) * 512],
                            start=(m == 0), stop=(m == 3))
                    ob = out_p.tile([128, 512], f32, name="ob", tag="ob")
                    nc.vector.tensor_copy(ob, po)
                    t0 = (qb * 4 + ti) * 128
                    nc.sync.dma_start(
                        out=out_d[t0:t0 + 128, nb * 512:(nb + 1) * 512], in_=ob)
                for ti in range(4):
                    for nb in range(2):
                        units.append(lambda ti=ti, nb=nb: op_unit(ti, nb))
                return units

            # ---------- attention pair tasks + phase driver ----------
            AluAdd = mybir.AluOpType.add
            spill_dram = {}
            d1_cur = {}

            def att_pair(qb, m, part, pump):
                """Emit one pair's k-loop. part: None=full, "A"=k<12 (spill),
                "B"=k>=12 (merge with spilled partial)."""
                nk = 4 * (qb + 1)
                k0, k1 = {"A": (0, 12), "B": (12, nk), None: (0, nk)}[part]
                pvp = ps_pv.tile([65, 1024], f32, name="pv", tag="pv")

                def pv_mms(pk, pt, stop):
                    nc.tensor.matmul(
                        pvp[:, 0:512],
                        vaug[pk][:, (2 * m) * 65:(2 * m + 1) * 65],
                        pt[:, 0:512], start=(pk == k0), stop=stop)
                    nc.tensor.matmul(
                        pvp[:, 512:1024],
                        vaug[pk][:, (2 * m + 1) * 65:(2 * m + 2) * 65],
                        pt[:, 512:1024], start=(pk == k0), stop=stop)

                prev = None
                for kti in range(k0, k1):
                    sp = ps_s.tile([128, 1024], f32, name="s", tag="s")
                    nc.tensor.matmul(
                        sp[:, 0:512],
                        kt[m][0:64, kti * 128:(kti + 1) * 128],
                        qt_cur[qb][m][0:64, :], start=True, stop=True)
                    nc.tensor.matmul(
                        sp[:, 512:1024],
                        kt[m][64:128, kti * 128:(kti + 1) * 128],
                        qt_cur[qb][m][64:128, :], start=True, stop=True)
                    ppt = p_p.tile([128, 1024], f32r, name="p", tag="p")
                    p3 = ppt.rearrange("p (h w) -> p h w", w=512)
                    s3 = sp.rearrange("p (h w) -> p h w", w=512)
                    j = kti - 4 * qb
                    if j < 0:
                        nc.scalar.activation(p3, s3, Exp, scale=0.125)
                    else:
                        w0 = 128 * j
                        nc.scalar.activation(
                            p3[:, :, w0:512], s3[:, :, w0:512], Exp, scale=0.125)
                        for h in range(2):
                            if j > 0:
                                nc.vector.tensor_copy(
                                    ppt[:, h * 512:h * 512 + w0], zerosr[:, 0:w0])
                            nc.vector.tensor_mul(
                                ppt[:, h * 512 + w0:h * 512 + w0 + 128],
                                ppt[:, h * 512 + w0:h * 512 + w0 + 128],
                                dstrip)
                    if prev is not None:
                        pv_mms(*prev, stop=False)
                    prev = (kti, ppt)
                    pump()
                pv_mms(*prev, stop=True)

                if part == "A":
                    st = spill_p.tile([65, 1024], f32, name="spst", tag="spst")
                    nc.vector.tensor_copy(st, pvp)
                    dsp = dram_p.tile([65, 1024], f32, name=f"dsp{m}", tag=f"dsp{m}")
                    nc.sync.dma_start(out=dsp, in_=st)
                    spill_dram[m] = dsp
                    return
                atm = at_p.tile([128, 512], f32r, name=f"at{m}", tag=f"at{m}")
                ats_cur[qb][m] = atm
                d1 = d1_cur[qb]
                if part == "B":
                    st = spill_p.tile([65, 1024], f32, name="spst", tag="spst")
                    nc.sync.dma_start(out=st, in_=spill_dram[m])
                    nc.vector.tensor_tensor(
                        atm[0:64, :], pvp[0:64, 0:512], st[0:64, 0:512], AluAdd)
                    nc.vector.tensor_tensor(
                        atm[64:128, :], pvp[0:64, 512:1024],
                        st[0:64, 512:1024], AluAdd)
                    for h in range(2):
                        dn = den_p.tile([1, 512], f32, name="dn", tag="dn")
                        nc.vector.tensor_tensor(
                            dn, pvp[64:65, h * 512:(h + 1) * 512],
                            st[64:65, h * 512:(h + 1) * 512], AluAdd)
                        nc.sync.dma_start(
                            out=d1[2 * m + h:2 * m + h + 1, :], in_=dn)
                else:
                    for h in range(2):
                        dn = den_p.tile([1, 512], f32, name="dn", tag="dn")
                        nc.vector.tensor_copy(
                            dn, pvp[64:65, h * 512:(h + 1) * 512])
                        nc.sync.dma_start(
                            out=d1[2 * m + h:2 * m + h + 1, :], in_=dn)
                    nc.vector.tensor_copy(atm[0:64, :], pvp[0:64, 0:512])
                    nc.vector.tensor_copy(atm[64:128, :], pvp[0:64, 512:1024])

            def normalize(qb):
                d1 = d1_cur[qb]
                den128 = rec_p.tile([128, 32], f32, name="den128", tag="den128")
                nc.sync.dma_start(
                    out=den128,
                    in_=d1.rearrange("i w -> (i w)").rearrange("(p c) -> p c", c=32))
                rec128 = rec_p.tile([128, 32], f32, name="rec128", tag="rec128")
                nc.vector.reciprocal(rec128, den128)
                d2 = dram_p.tile([8, 512], f32, name="d2", tag="d2")
                nc.sync.dma_start(
                    out=d2.rearrange("i w -> (i w)").rearrange("(p c) -> p c", c=32),
                    in_=rec128)
                for m in range(4):
                    bcs = bcs_p.tile([128, 512], f32, name="bcs", tag="bcs")
                    for h in range(2):
                        nc.sync.dma_start(
                            out=bcs[h * 64:(h + 1) * 64, :],
                            in_=d2[2 * m + h:2 * m + h + 1, :].partition_broadcast(64))
                    nc.vector.tensor_mul(ats_cur[qb][m], ats_cur[qb][m], bcs)

            def run_phase(tasks, fillers, n_units):
                """tasks: closures taking pump(); fillers pumped proportionally."""
                nf = len(fillers)
                state = {"fi": 0, "ai": 0}

                def pump():
                    state["ai"] += 1
                    while state["fi"] * n_units < state["ai"] * nf \
                            and state["fi"] < nf:
                        fillers[state["fi"]]()
                        state["fi"] += 1
                for t in tasks:
                    t(pump)
                while state["fi"] < nf:
                    fillers[state["fi"]]()
                    state["fi"] += 1

            # ---------------- emission schedule ----------------
            for u in qkv_units(0):
                u()
            for qb in range(NQ):
                ats_cur[qb] = [None] * 4
                d1_cur[qb] = dram_p.tile([8, 512], f32, name="d1", tag="d1")

            def phase_tasks(qb, part):
                def mk(m):
                    def t(pump):
                        att_pair(qb, m, part, pump)
                    return t
                return [mk(m) for m in range(4)]

            def norm_task(qb):
                def t(pump):
                    normalize(qb)
                return t

            # phase 0: att(0) + qkv(1)
            run_phase(phase_tasks(0, None) + [norm_task(0)],
                      qkv_units(1), 16)
            # phase 1: att(1) + op(0) + qkv(2)
            run_phase(phase_tasks(1, None) + [norm_task(1)],
                      outproj_units(0) + qkv_units(2), 32)
            # phase 2: att(2) + op(1) + qkv(3)
            run_phase(phase_tasks(2, None) + [norm_task(2)],
                      outproj_units(1) + qkv_units(3), 48)
            # phase 3: att(3) with deferred out_proj(2) as PE filler
            run_phase(phase_tasks(3, None) + [norm_task(3)],
                      outproj_units(2), 64)
            for u in outproj_units(NQ - 1):
                u()
    nc.finalize()
    return nc


_NC_CACHE = {}


def _get_nc():
    if "nc" not in _NC_CACHE:
        _NC_CACHE["nc"] = build_nc()
    return _NC_CACHE["nc"]


def _make_in_maps(x, w_qkv, w_out, attn_mask):
    x = np.asarray(x, dtype=np.float32)
    w_qkv = np.asarray(w_qkv, dtype=np.float32)
    w_out = np.asarray(w_out, dtype=np.float32)
    am = np.asarray(attn_mask)
    in_maps = []
    for c in range(NCORES):
        b, hg = c // 2, c % 2
        wqk_c = np.ascontiguousarray(np.concatenate(
            [w_qkv[:, hg * CQ:(hg + 1) * CQ],
             w_qkv[:, DIM + hg * CQ:DIM + (hg + 1) * CQ]], axis=1))
        wv_c = np.ascontiguousarray(w_qkv[:, 2 * DIM + hg * CQ:2 * DIM + (hg + 1) * CQ])
        wo_c = np.ascontiguousarray(w_out[hg * CQ:(hg + 1) * CQ, :])
        mv_c = np.ascontiguousarray(
            am[b].astype(np.float32).reshape(NT, 128).T)
        in_maps.append({
            "x": np.ascontiguousarray(x[b]),
            "wqk": wqk_c,
            "wv": wv_c,
            "wo": wo_c,
            "maskv": mv_c,
        })
    return in_maps


def run(x, w_qkv, w_out, attn_mask, trace=False):
    nc = _get_nc()
    in_maps = _make_in_maps(x, w_qkv, w_out, attn_mask)
    res = run_bass_kernel_spmd(nc, in_maps, list(range(NCORES)), trace=trace)
    outs = [res.results[c]["out"] for c in range(NCORES)]
    full = np.stack([outs[2 * b] + outs[2 * b + 1] for b in range(B)], axis=0)
    return full.astype(np.float32), res


def kernel(x, w_qkv, w_out, attn_mask):
    full, _ = run(x, w_qkv, w_out, attn_mask, trace=False)
    return full
